# revision 1
# baseline (speedup 1.0000x reference)
"""3-layer GCN (GCNConv x3 + Linear + to_dense_batch) on 8 TRN2 NeuronCores.

Strategy (graph/data parallel, per sharding hint):
  - Nodes are assigned to 8 cores (12800 each), 128 dst-tiles of 100 nodes per
    core. Edges are partitioned by destination-node owner.
  - GCNConv is computed as aggregate-then-transform:
        agg[d] = sum_{e: dst=e} coef_e * h[src_e] + dinv[d]^2 * h[d]
        h'     = relu(agg @ W + b)
    The per-tile segment-sum is a dense matmul on the PE: for each
    (dst-tile, src-quarter) a [128 edge-slots x 128 dst] coefficient matrix S
    (host-built, bf16) multiplies gathered source rows G (indirect DMA).
  - Source features are gathered from 4 "stage" tables (one per quarter of the
    node space). After each layer, every core AllGathers its produced shard
    into the stage tables, chunked 4x so collectives overlap with compute.
  - The final FC and to_dense_batch scatter: FC on-device, scatter on host
    (pure index movement).
"""

import numpy as np
import ml_dtypes

# ---------------------------------------------------------------- constants
N, E, B = 102400, 409600, 2048
F0, OUT = 78, 200
FIN = [78, 78, 156]          # aggregation width per layer
FOUT = [78, 156, 312]        # conv output width per layer
FOUT_PAD = [256, 256, 312]   # psum free-dim padding (>=256 for full-rate f32r)
OUT_PAD = 256
NCORES = 8
SHARD = N // NCORES          # 12800 nodes per core
TILE_N = 100                 # nodes per dst tile
TILES = SHARD // TILE_N      # 128 tiles per core
NQ = 4                       # stage tables == AllGather chunks
AGB = TILES // NQ            # 32 tiles per AG block
TQ_PC = AGB * TILE_N         # 3200 rows per core per block
TROWS = NCORES * TQ_PC       # 25600 rows per stage table
SCT = 8                      # tiles per superchunk (gather granularity)
NSC = TILES // SCT           # 16 superchunks per core
PCT = 4                      # tiles per psum chunk
NGROUPS = TILES * NQ         # 512 S-blocks per core
SC_COLS = 5 * SCT            # logical idx cols per superchunk (4 q + self) * 8
IDXCOLS = NSC * SC_COLS      # 640 (logical, for host sims)
ELEM = [128, 128, 256]       # padded table row elems per layer (bf16, %256B)
CALL_IDX = SCT * 128         # 1024 idxs per gather call
IDX16_COLS = NSC * 5 * (CALL_IDX // 16)   # 5120 int16 cols

BF16 = ml_dtypes.bfloat16


# ---------------------------------------------------------------- host prep
def _host_prep(x, edge_index, W1, b1, W2, b2, W3, b3, Wfc, bfc):
    x = np.asarray(x, np.float32)
    src = np.asarray(edge_index[0], np.int64)
    dst = np.asarray(edge_index[1], np.int64)

    deg = (np.bincount(dst, minlength=N) + 1.0).astype(np.float32)
    dinv = (1.0 / np.sqrt(deg)).astype(np.float32)

    nodes = np.arange(N)
    core_of = nodes // SHARD
    tloc = (nodes % SHARD) // TILE_N
    slot = (nodes % SHARD) % TILE_N
    qblk = tloc // AGB            # AG block of each node; preserved by repair

    # per-node in-degree split by the src's AG block
    eq = qblk[src]
    indegq = np.zeros((N, NQ), np.int64)
    np.add.at(indegq, (dst, eq), 1)

    # Repair: every (tile, src-quarter) cell must hold <= 128 edges.  Swap
    # nodes between tiles of the same (core, AG-block) — this keeps every
    # node's AG block (hence all edge quarters) unchanged.
    gt = core_of * TILES + tloc
    for _ in range(2000):
        cnt = np.zeros((NCORES * TILES, NQ), np.int64)
        np.add.at(cnt, gt, indegq)
        bad = np.argwhere(cnt > 128)
        if len(bad) == 0:
            break
        g_bad, q_bad = int(bad[0][0]), int(bad[0][1])
        core, t_bad = g_bad // TILES, g_bad % TILES
        blk = t_bad // AGB
        members = np.where(gt == g_bad)[0]
        a = members[np.argmax(indegq[members, q_bad])]
        cands = core * TILES + blk * AGB + np.arange(AGB)
        cands = cands[cands != g_bad]
        done = False
        for g2 in cands[np.argsort(cnt[cands, q_bad])]:
            mem2 = np.where(gt == g2)[0]
            for b_ in mem2[np.argsort(indegq[mem2, q_bad])][:4]:
                d = indegq[a] - indegq[b_]
                if d[q_bad] <= 0:
                    continue
                new_src = cnt[g_bad] - d
                new_dst = cnt[g2] + d
                # must strictly reduce the bad cell, and not create overflow
                # anywhere that wasn't already at least as bad
                if (np.all(new_dst <= 128)
                        and np.all((new_src <= 128) | (new_src < cnt[g_bad]))):
                    tloc[a], tloc[b_] = tloc[b_], tloc[a]
                    slot[a], slot[b_] = slot[b_], slot[a]
                    gt[a], gt[b_] = gt[b_], gt[a]
                    done = True
                    break
            if done:
                break
        if not done:
            raise RuntimeError("tile repair: no donor found")
    else:
        raise RuntimeError("tile repair did not converge")

    trow = core_of * TQ_PC + (tloc % AGB) * TILE_N + slot   # row in stage table

    # ---- per-edge bookkeeping
    e_core = core_of[dst]
    e_tile = tloc[dst]
    e_q = qblk[src]
    e_dslot = slot[dst]
    e_trow = trow[src]
    coef = (dinv[src] * dinv[dst]).astype(np.float32)

    cellkey = (e_core * TILES + e_tile) * NQ + e_q
    order = np.argsort(cellkey, kind="stable")
    ck = cellkey[order]
    starts = np.searchsorted(ck, np.arange(NCORES * TILES * NQ))
    pos = np.arange(E) - starts[ck]
    assert pos.max() < 128, "cell overflow after repair"

    c_, t_, q_ = e_core[order], e_tile[order], e_q[order]
    S = np.zeros((NCORES, 128, NGROUPS * 128), np.float32)
    S[c_, pos, (t_ * NQ + q_) * 128 + e_dslot[order]] = coef[order]

    idxs = np.zeros((NCORES, 128, IDXCOLS), np.int32)
    col = (t_ // SCT) * SC_COLS + q_ * SCT + (t_ % SCT)
    idxs[c_, pos, col] = e_trow[order]

    # self columns: partition p == slot p
    col_s = (tloc // SCT) * SC_COLS + 4 * SCT + (tloc % SCT)
    idxs[core_of, slot, col_s] = trow

    coefself = np.zeros((NCORES, 128, TILES), np.float32)
    coefself[core_of, slot, tloc] = dinv * dinv

    idxs_logical = idxs
    # dma_gather int16 index stream: per call (sc, k), linear slot
    # i = j*128 + p holds idxs_logical[p, base+k*8+j]; wrapped in 16
    # partitions ([c, s] = lin[s*16+c]) and replicated across the 8 Q7 pairs.
    idx16 = np.zeros((NCORES, 128, IDX16_COLS), np.int16)
    for c in range(NCORES):
        for sc in range(NSC):
            for k in range(5):
                cb = sc * SC_COLS + k * SCT
                lin = idxs_logical[c][:, cb:cb + SCT].T.reshape(-1)  # [1024]
                wrapped = lin.reshape(CALL_IDX // 16, 16).T          # [16, 64]
                o = (sc * 5 + k) * (CALL_IDX // 16)
                idx16[c][:, o:o + CALL_IDX // 16] = np.tile(wrapped, (8, 1))

    # layer-1 gather tables (replicated on every core), rows padded to 256B
    xq = np.zeros((NQ, TROWS, ELEM[0]), BF16)
    xq[qblk, trow, :F0] = x.astype(BF16)

    # augmented weights (bias folded as an extra contraction row)
    def aug(W, b, fout_pad):
        W = np.asarray(W, np.float32)
        b = np.asarray(b, np.float32)
        k, f = W.shape
        A = np.zeros((k + 1, fout_pad), np.float32)
        A[:k, :f] = W
        A[k, :f] = b
        return A

    w1a = aug(W1, b1, FOUT_PAD[0])               # [79, 256]
    w2a = aug(W2, b2, FOUT_PAD[1])               # [79, 256]
    w3a = aug(W3, b3, FOUT_PAD[2])               # [157, 312]
    wfca = aug(Wfc, bfc, OUT_PAD)                # [313, 256]

    out_row = core_of * SHARD + tloc * TILE_N + slot    # node -> output row

    return dict(
        S=S.astype(BF16),
        idx16=idx16,
        idxs_logical=idxs_logical,
        coefself=coefself,
        xq=xq,
        w1a=w1a, w2a=w2a,
        w3a0=np.ascontiguousarray(w3a[:128]), w3a1=np.ascontiguousarray(w3a[128:]),
        wfc0=np.ascontiguousarray(wfca[:128]),
        wfc1=np.ascontiguousarray(wfca[128:256]),
        wfc2=np.ascontiguousarray(wfca[256:]),
        out_row=out_row,
    )


# ------------------------------------------------------------- bass program
QUEUES = 1

_TILE_KEEPALIVE = []


def _tctile(tc, shape, dtype, name):
    t, free = tc.tile(shape, dtype, name=name)
    _TILE_KEEPALIVE.append(free)
    return t

def _build_program(nocc=False):
    import concourse.bacc as bacc
    import concourse.bass as bass
    import concourse.mybir as mybir
    import concourse.tile as tile
    from concourse.masks import make_identity

    dt = mybir.dt
    nc = bacc.Bacc("TRN2", target_bir_lowering=False, debug=False,
                   num_devices=NCORES, num_swdge_queues=QUEUES)

    xq_d = [nc.dram_tensor(f"xq{q}", [TROWS, ELEM[0]], dt.bfloat16,
                           kind="ExternalInput") for q in range(NQ)]
    s_d = nc.dram_tensor("s_hbm", [128, NGROUPS * 128], dt.bfloat16,
                         kind="ExternalInput")
    idx_d = nc.dram_tensor("idx_hbm", [128, IDX16_COLS], dt.int16,
                           kind="ExternalInput")
    cself_d = nc.dram_tensor("cself_hbm", [128, TILES], dt.float32,
                             kind="ExternalInput")
    w1_d = nc.dram_tensor("w1a", [79, FOUT_PAD[0]], dt.float32r, kind="ExternalInput")
    w2_d = nc.dram_tensor("w2a", [79, FOUT_PAD[1]], dt.float32r, kind="ExternalInput")
    w3a0_d = nc.dram_tensor("w3a0", [128, FOUT_PAD[2]], dt.float32r, kind="ExternalInput")
    w3a1_d = nc.dram_tensor("w3a1", [29, FOUT_PAD[2]], dt.float32r, kind="ExternalInput")
    wfc0_d = nc.dram_tensor("wfc0", [128, OUT_PAD], dt.float32r, kind="ExternalInput")
    wfc1_d = nc.dram_tensor("wfc1", [128, OUT_PAD], dt.float32r, kind="ExternalInput")
    wfc2_d = nc.dram_tensor("wfc2", [57, OUT_PAD], dt.float32r, kind="ExternalInput")
    out_d = nc.dram_tensor("out_fc", [SHARD, OUT], dt.float32, kind="ExternalOutput")
    dbg = {}
    if DEBUG:
        dbg["dbg_g0"] = nc.dram_tensor("dbg_g0", [128, SCT, ELEM[0]], dt.bfloat16,
                                       kind="ExternalOutput")
        dbg["dbg_gs"] = nc.dram_tensor("dbg_gs", [128, SCT, ELEM[0]], dt.bfloat16,
                                       kind="ExternalOutput")
        dbg["dbg_idx"] = nc.dram_tensor("dbg_idx", [128, 320], dt.int16,
                                        kind="ExternalOutput")
        dbg["dbg_s"] = nc.dram_tensor("dbg_s", [128, 512], dt.bfloat16,
                                      kind="ExternalOutput")
        dbg["dbg_m"] = nc.dram_tensor("dbg_m", [128, FIN[0]], dt.float32,
                                      kind="ExternalOutput")
        dbg["dbg_at"] = nc.dram_tensor("dbg_at", [79, 128], dt.float32r,
                                       kind="ExternalOutput")
        for q in range(NQ):
            dbg[f"dbg_h1loc_{q}"] = nc.dram_tensor(
                f"dbg_h1loc_{q}", [TQ_PC, ELEM[1]], dt.bfloat16, kind="ExternalOutput")
            dbg[f"dbg_h1st_{q}"] = nc.dram_tensor(
                f"dbg_h1st_{q}", [TROWS, ELEM[1]], dt.bfloat16, kind="ExternalOutput")

    f32r = dt.float32r
    RG = [list(range(NCORES))]

    with tile.TileContext(nc) as tc:
        # ------------------------------------------------ DRAM intermediates
        with tc.tile_pool(name="dram", bufs=1, space="DRAM") as dpool:
            h_loc = {}
            h_stage = {}
            for lay in (0, 1):
                f = ELEM[lay + 1]
                for q in range(NQ):
                    h_loc[(lay, q)] = dpool.tile(
                        [TQ_PC, f], dt.bfloat16, name=f"hloc{lay}_{q}")
                    h_stage[(lay, q)] = dpool.tile(
                        [TROWS, f], dt.bfloat16, addr_space="Shared",
                        name=f"hstage{lay}_{q}")

            # -------------------------------------------- resident SBUF data
            s_sb = _tctile(tc, [128, NGROUPS * 128], dt.bfloat16, name="s_sb")
            idx_sb = _tctile(tc, [128, IDX16_COLS], dt.int16, name="idx_sb")
            cself_sb = _tctile(tc, [128, TILES], dt.float32, name="cself_sb")
            w1_sb = _tctile(tc, [79, FOUT_PAD[0]], dt.float32r, name="w1_sb")
            w2_sb = _tctile(tc, [79, FOUT_PAD[1]], dt.float32r, name="w2_sb")
            w3a0_sb = _tctile(tc, [128, FOUT_PAD[2]], dt.float32r, name="w3a0_sb")
            w3a1_sb = _tctile(tc, [29, FOUT_PAD[2]], dt.float32r, name="w3a1_sb")
            wfc0_sb = _tctile(tc, [128, OUT_PAD], dt.float32r, name="wfc0_sb")
            wfc1_sb = _tctile(tc, [128, OUT_PAD], dt.float32r, name="wfc1_sb")
            wfc2_sb = _tctile(tc, [57, OUT_PAD], dt.float32r, name="wfc2_sb")
            ident = _tctile(tc, [128, 128], dt.float32, name="ident")

            nc.sync.dma_start(s_sb[:], s_d[:])
            nc.sync.dma_start(idx_sb[:], idx_d[:])
            nc.sync.dma_start(cself_sb[:], cself_d[:])
            nc.sync.dma_start(w1_sb[:], w1_d[:])
            nc.sync.dma_start(w2_sb[:], w2_d[:])
            nc.sync.dma_start(w3a0_sb[:], w3a0_d[:])
            nc.sync.dma_start(w3a1_sb[:], w3a1_d[:])
            nc.sync.dma_start(wfc0_sb[:], wfc0_d[:])
            nc.sync.dma_start(wfc1_sb[:], wfc1_d[:])
            nc.sync.dma_start(wfc2_sb[:], wfc2_d[:])
            make_identity(nc, ident[:])

            # ------------------------------------------------- working pools
            gpool = tc.alloc_tile_pool(name="gpool", bufs=2)
            wpool = tc.alloc_tile_pool(name="wpool", bufs=3)
            pm_pool = tc.alloc_tile_pool(name="pm", bufs=PCT, space="PSUM")
            ptr_pool = tc.alloc_tile_pool(name="ptr", bufs=2, space="PSUM")
            ph_pool = tc.alloc_tile_pool(name="ph", bufs=1, space="PSUM")
            pfc_pool = tc.alloc_tile_pool(name="pfc", bufs=1, space="PSUM")

            for lay in range(3):
                fin, fout, fpad = FIN[lay], FOUT[lay], FOUT_PAD[lay]
                if lay == 0:
                    tabs = [t.ap() for t in xq_d]
                else:
                    tabs = [h_stage[(lay - 1, q)] for q in range(NQ)]

                elem = ELEM[lay]
                for sc in range(NSC):
                    Q = sc // (NSC // NQ)
                    ibase = sc * 5 * (CALL_IDX // 16)
                    # ---- gathers: 4 source quarters + self
                    gts = []
                    for qq in range(NQ):
                        g = gpool.tile([128, SCT, elem], dt.bfloat16,
                                       tag=f"g{qq}", name=f"g{lay}_{sc}_{qq}")
                        nc.gpsimd.dma_gather(
                            g[:, :, :],
                            tabs[qq][:, :],
                            idx_sb[:, ibase + qq * (CALL_IDX // 16):
                                   ibase + (qq + 1) * (CALL_IDX // 16)],
                            CALL_IDX, CALL_IDX, elem,
                            queue_num=(sc * 5 + qq) % QUEUES,
                        )
                        gts.append(g)
                    gs = gpool.tile([128, SCT, elem], dt.bfloat16,
                                    tag="gs", name=f"gs{lay}_{sc}")
                    nc.gpsimd.dma_gather(
                        gs[:, :, :],
                        tabs[Q][:, :],
                        idx_sb[:, ibase + 4 * (CALL_IDX // 16):
                               ibase + 5 * (CALL_IDX // 16)],
                        CALL_IDX, CALL_IDX, elem,
                        queue_num=(sc * 5 + 4) % QUEUES,
                    )

                    if DEBUG and lay == 0 and sc == 0:
                        nc.sync.dma_start(dbg["dbg_g0"][:, :, :], gts[0][:, :, :])
                        nc.sync.dma_start(dbg["dbg_gs"][:, :, :], gs[:, :, :])
                        nc.sync.dma_start(dbg["dbg_idx"][:, :], idx_sb[:, :320])
                        nc.sync.dma_start(dbg["dbg_s"][:, :], s_sb[:, :512])

                    for pc in range(SCT // PCT):
                        pms = []
                        for i in range(PCT):
                            pm = pm_pool.tile([128, fin], dt.float32,
                                              tag="pm", name=f"pm{lay}_{sc}_{pc}_{i}")
                            pms.append(pm)
                        # ---- aggregation matmuls
                        for qq in range(NQ):
                            for i in range(PCT):
                                j = pc * PCT + i
                                t = sc * SCT + j
                                g_idx = t * NQ + qq
                                nc.tensor.matmul(
                                    pms[i][:, :],
                                    lhsT=s_sb[:, g_idx * 128:(g_idx + 1) * 128],
                                    rhs=gts[qq][:, j, :fin],
                                    start=(qq == 0),
                                    stop=(qq == NQ - 1),
                                )
                        # ---- finish each tile
                        for i in range(PCT):
                            j = pc * PCT + i
                            t = sc * SCT + j
                            # self term: tmp = coefself[:, t] * G_self
                            tmp = wpool.tile([128, fin], dt.float32, tag="tmp",
                                             name=f"tmp{lay}_{t}")
                            nc.vector.tensor_scalar_mul(
                                tmp[:], gs[:, j, :fin], cself_sb[:, t:t + 1])
                            a_sb = wpool.tile([128, fin + 1], dt.float32, tag="a",
                                              name=f"a{lay}_{t}")
                            nc.vector.tensor_tensor(
                                out=a_sb[:, :fin], in0=pms[i][:, :], in1=tmp[:],
                                op=mybir.AluOpType.add)
                            nc.vector.memset(a_sb[:, fin:fin + 1], 1.0)
                            if DEBUG and lay == 0 and t == 0:
                                mcp = wpool.tile([128, fin], dt.float32,
                                                 tag="mcp", name="mcp_dbg")
                                nc.vector.tensor_copy(mcp[:], pms[i][:, :])
                                nc.sync.dma_start(dbg["dbg_m"][:, :], mcp[:])

                            # transpose a -> aT (ones col becomes bias row)
                            if fin == 78:
                                at = wpool.tile([79, 128], dt.float32r, tag="at0",
                                                name=f"at{lay}_{t}")
                                ptr = ptr_pool.tile([79, 128], dt.float32,
                                                    tag="ptr", name=f"ptr{lay}_{t}")
                                nc.tensor.transpose(ptr[:79, :], a_sb[:, :79], ident[:])
                                nc.scalar.copy(at[:79, :], ptr[:79, :])
                                if DEBUG and lay == 0 and t == 0:
                                    nc.sync.dma_start(dbg["dbg_at"][:, :], at[:79, :])
                                ats = [(at, 79)]
                            else:
                                at0 = wpool.tile([128, 128], dt.float32r, tag="at0",
                                                 name=f"at0_{lay}_{t}")
                                at1 = wpool.tile([29, 128], dt.float32r, tag="at1",
                                                 name=f"at1_{lay}_{t}")
                                ptr0 = ptr_pool.tile([128, 128], dt.float32,
                                                     tag="ptr", name=f"ptr0_{lay}_{t}")
                                nc.tensor.transpose(ptr0[:], a_sb[:, :128], ident[:])
                                nc.scalar.copy(at0[:, :], ptr0[:])
                                ptr1 = ptr_pool.tile([29, 128], dt.float32,
                                                     tag="ptr", name=f"ptr1_{lay}_{t}")
                                nc.tensor.transpose(ptr1[:29, :], a_sb[:, 128:157], ident[:])
                                nc.scalar.copy(at1[:29, :], ptr1[:29, :])
                                ats = [(at0, 128), (at1, 29)]

                            # W matmul (f32r)
                            ph = ph_pool.tile([128, fpad], dt.float32, tag="ph",
                                              name=f"ph{lay}_{t}")
                            if lay == 0:
                                wtiles = [(w1_sb, 79)]
                            elif lay == 1:
                                wtiles = [(w2_sb, 79)]
                            else:
                                wtiles = [(w3a0_sb, 128), (w3a1_sb, 29)]
                            nk = len(ats)
                            for k, ((at_t, kp), (w_t, kp2)) in enumerate(
                                    zip(ats, wtiles)):
                                assert kp == kp2
                                nc.tensor.matmul(
                                    ph[:, :],
                                    lhsT=at_t[:kp, :],
                                    rhs=w_t[:kp, :],
                                    start=(k == 0), stop=(k == nk - 1),
                                )

                            if lay < 2:
                                # relu -> bf16, store to local AG shard
                                hb = wpool.tile([128, fout], dt.bfloat16, tag="hb",
                                                name=f"hb{lay}_{t}")
                                nc.scalar.activation(
                                    hb[:], ph[:, :fout],
                                    mybir.ActivationFunctionType.Relu)
                                tq = t // AGB
                                r0 = (t % AGB) * TILE_N
                                nc.sync.dma_start(
                                    h_loc[(lay, tq)][r0:r0 + TILE_N, :fout],
                                    hb[:TILE_N, :])
                            else:
                                # relu (f32) -> transpose -> FC -> out rows
                                h3 = wpool.tile([128, 313], dt.float32, tag="h3",
                                                name=f"h3_{t}")
                                nc.scalar.activation(
                                    h3[:, :312], ph[:, :312],
                                    mybir.ActivationFunctionType.Relu)
                                nc.vector.memset(h3[:, 312:313], 1.0)
                                f0 = wpool.tile([128, 128], dt.float32r, tag="f0",
                                                name=f"f0_{t}")
                                f1 = wpool.tile([128, 128], dt.float32r, tag="f1",
                                                name=f"f1_{t}")
                                f2 = wpool.tile([57, 128], dt.float32r, tag="f2",
                                                name=f"f2_{t}")
                                p0 = ptr_pool.tile([128, 128], dt.float32,
                                                   tag="ptr", name=f"p0_{t}")
                                nc.tensor.transpose(p0[:], h3[:, 0:128], ident[:])
                                nc.scalar.copy(f0[:, :], p0[:])
                                p1 = ptr_pool.tile([128, 128], dt.float32,
                                                   tag="ptr", name=f"p1_{t}")
                                nc.tensor.transpose(p1[:], h3[:, 128:256], ident[:])
                                nc.scalar.copy(f1[:, :], p1[:])
                                p2 = ptr_pool.tile([57, 128], dt.float32,
                                                   tag="ptr", name=f"p2_{t}")
                                nc.tensor.transpose(p2[:57, :], h3[:, 256:313], ident[:])
                                nc.scalar.copy(f2[:57, :], p2[:57, :])
                                pfc = pfc_pool.tile([128, OUT_PAD], dt.float32,
                                                    tag="pfc", name=f"pfc_{t}")
                                for k, (ft, kp, wt) in enumerate([
                                        (f0, 128, wfc0_sb), (f1, 128, wfc1_sb),
                                        (f2, 57, wfc2_sb)]):
                                    nc.tensor.matmul(
                                        pfc[:, :],
                                        lhsT=ft[:kp, :],
                                        rhs=wt[:kp, :],
                                        start=(k == 0), stop=(k == 2),
                                    )
                                ob = wpool.tile([128, OUT], dt.float32, tag="ob",
                                                name=f"ob_{t}")
                                nc.vector.tensor_copy(ob[:], pfc[:, :OUT])
                                nc.sync.dma_start(
                                    out_d[t * TILE_N:(t + 1) * TILE_N, :],
                                    ob[:TILE_N, :])

                    # ---- chunked AllGather after each 32-tile block
                    if lay < 2 and sc % (NSC // NQ) == (NSC // NQ) - 1 and nocc:
                        nc.sync.dma_start(h_stage[(lay, Q)][:TQ_PC, :],
                                          h_loc[(lay, Q)][:, :])
                    elif lay < 2 and sc % (NSC // NQ) == (NSC // NQ) - 1:
                        nc.gpsimd.collective_compute(
                            "AllGather",
                            mybir.AluOpType.bypass,
                            replica_groups=RG,
                            ins=[h_loc[(lay, Q)][:, :].opt()],
                            outs=[h_stage[(lay, Q)][:, :].opt()],
                        )

            if DEBUG:
                for q in range(NQ):
                    nc.sync.dma_start(dbg[f"dbg_h1loc_{q}"][:, :],
                                      h_loc[(0, q)][:, :])
                    nc.sync.dma_start(dbg[f"dbg_h1st_{q}"][:, :],
                                      h_stage[(0, q)][:, :])

            for _p in (pfc_pool, ph_pool, ptr_pool, pm_pool, wpool, gpool):
                _p.release()

    nc.compile()
    return nc


# ------------------------------------------------------------------- kernel
_CACHED = {}
TRACE = False
DEBUG = False
LAST_RESULT = None
LAST_IN_MAPS = None
LAST_NC = None


def kernel(**inputs):
    x = np.asarray(inputs["x"], np.float32)
    edge_index = np.asarray(inputs["edge_index"], np.int32)
    batch = np.asarray(inputs["batch"], np.int64)
    max_num = int(np.asarray(inputs["max_num"]))

    prep = _host_prep(
        x, edge_index,
        inputs["W1"], inputs["b1"], inputs["W2"], inputs["b2"],
        inputs["W3"], inputs["b3"], inputs["Wfc"], inputs["bfc"])

    if "nc" not in _CACHED:
        _CACHED["nc"] = _build_program()
    nc = _CACHED["nc"]

    from concourse.bass_utils import run_bass_kernel_spmd

    in_maps = []
    for c in range(NCORES):
        m = {
            "s_hbm": np.ascontiguousarray(prep["S"][c]),
            "idx_hbm": np.ascontiguousarray(prep["idx16"][c]),
            "cself_hbm": np.ascontiguousarray(prep["coefself"][c]),
            "w1a": prep["w1a"], "w2a": prep["w2a"],
            "w3a0": prep["w3a0"], "w3a1": prep["w3a1"],
            "wfc0": prep["wfc0"], "wfc1": prep["wfc1"], "wfc2": prep["wfc2"],
        }
        for q in range(NQ):
            m[f"xq{q}"] = np.ascontiguousarray(prep["xq"][q])
        in_maps.append(m)

    global LAST_RESULT, LAST_IN_MAPS, LAST_NC
    LAST_IN_MAPS, LAST_NC = in_maps, nc
    res = run_bass_kernel_spmd(nc, in_maps, core_ids=list(range(NCORES)),
                               trace=TRACE)
    LAST_RESULT = res
    shards = [res.results[c]["out_fc"] for c in range(NCORES)]
    h_rows = np.concatenate(shards, axis=0)          # [N, OUT] in table order
    h_nodes = h_rows[prep["out_row"]]                # [N, OUT] in node order

    # ---- to_dense_batch on host (pure scatter)
    counts = np.bincount(batch, minlength=B)
    starts = np.concatenate([[0], np.cumsum(counts)[:-1]])
    pos = np.arange(N) - starts[batch]
    valid = pos < max_num
    flat = np.where(valid, batch * max_num + pos, B * max_num)
    dense = np.zeros((B * max_num + 1, OUT), np.float32)
    dense[flat] = h_nodes
    return dense[:-1].reshape(B, max_num, OUT)

